# revision 1
# baseline (speedup 1.0000x reference)
"""Trainium2 Bass kernel for nn_AdaptiveModalityEncoder.

Reference computation (per row r of input_data [B, D]):
    sel[r] = selection_mask[r, modality_idx] > 0.5
    out[r] = sel[r] ? gelu(x[r] @ W1 + b1) @ W2 + b2 : 0

Strategy (moe_routing, data-parallel across 8 cores):
  - Shard batch B=16384 into 8 x 2048 rows; replicate weights.
  - Host computes per-shard selected-row index lists (routing metadata);
    each core device-side: indirect-DMA gathers ONLY the selected rows,
    PE-transposes them, runs the 2-layer MLP in bf16 (fp32 accumulate),
    and indirect-DMA scatters encoded rows back into a zero-filled
    output ("mask-gather-encode-scatter", no cross-core communication).
  - Unselected rows are zero-filled by scattering from a zero tile.
  - Compute skips ~50% of rows (mask is ~Bernoulli(0.5)).

Matmul layout: activations kept feature-major (X^T tiles [D_part, rows]);
L1: H^T = W1^T @ X^T (lhsT=W1 natural). L2 flips operands so output comes
out batch-major directly: OUT = (HT)^T @ W2 (lhsT=HT slices) -> natural
rows, scattered straight to DRAM. Only the input needs a PE transpose.
"""

import sys

sys.path.insert(0, "/opt/trn_rl_repo")

import numpy as np
import ml_dtypes

# Problem constants (hardcoded per harness contract).
B, D, H, O, K = 16384, 1024, 2048, 1024, 4
NCORES = 8
R = B // NCORES  # rows per core = 2048
P = 128
# Scatter-index sentinel: one past the last valid row, skipped via
# bounds_check. Must stay small — the DGE multiplies idx by the row stride
# in 32-bit, so a huge sentinel would overflow and alias a valid row.
OOB = R

_GRAPH_CACHE = {}


def _build_graph(NG, NGZ, EX, act="gelu"):
    """Build + compile the per-core Bass graph. NG = selected-row tiles
    (128 rows each), NGZ = zero-fill tiles, EX = borrowed-row slots appended
    to the shard (load balancing). Same graph runs on all 8 cores."""
    import concourse.bass as bass
    import concourse.mybir as mybir
    import concourse.tile as tile
    from concourse import bacc
    from concourse.masks import make_identity

    f32 = mybir.dt.float32
    bf16 = mybir.dt.bfloat16
    i32 = mybir.dt.int32
    act_fn = {
        "gelu": mybir.ActivationFunctionType.Gelu_apprx_tanh,
        "tanh": mybir.ActivationFunctionType.Tanh,  # CoreSim stand-in
    }[act]

    C = NG * P  # padded selected-row count
    NCOL = 2 * NG + NGZ  # idx columns: gather | scatter | zero

    nc = bacc.Bacc("TRN2", target_bir_lowering=False, debug=False, num_devices=NCORES)

    RX = R + EX  # shard rows + borrowed-row block
    x_d = nc.dram_tensor("x", [RX, D], bf16, kind="ExternalInput")
    w1_d = nc.dram_tensor("w1", [D, H], bf16, kind="ExternalInput")
    w2_d = nc.dram_tensor("w2", [H, O], bf16, kind="ExternalInput")
    b1_d = nc.dram_tensor("b1p", [P, H // P], f32, kind="ExternalInput")
    b2_d = nc.dram_tensor("b2row", [1, O], bf16, kind="ExternalInput")
    idx_d = nc.dram_tensor("idx", [P, NCOL], i32, kind="ExternalInput")
    ident_d = nc.dram_tensor("identm", [P, P], bf16, kind="ExternalInput")
    out_d = nc.dram_tensor("out", [RX, O], f32, kind="ExternalOutput")

    KD = D // P  # 8 k-tiles for layer 1
    KH = H // P  # 16 k-tiles for layer 2
    NO = O // P  # 8 output column tiles

    # Column chunks of the gathered batch for L1 (PSUM bank = 512 fp32).
    chunks = []
    c0 = 0
    while c0 < C:
        w = min(512, C - c0)
        chunks.append((c0, w))
        c0 += w

    with tile.TileContext(nc) as tc:
        with (
            tc.tile_pool(name="w1pool", bufs=KD) as w1pool,
            tc.tile_pool(name="w2pool", bufs=KH) as w2pool,
            tc.tile_pool(name="xtp", bufs=2 * KD) as xtp,
            tc.tile_pool(name="htp", bufs=2 * KH) as htp,
            tc.tile_pool(name="outp", bufs=2) as outp,
            tc.tile_pool(name="const", bufs=1) as constp,
            tc.tile_pool(name="pst", bufs=2, space="PSUM") as pst,  # transposes
            tc.tile_pool(name="ps1", bufs=3, space="PSUM") as ps1,  # layer 1
            tc.tile_pool(name="ps2", bufs=3, space="PSUM") as ps2,  # layer 2
        ):
            # ---- constants / small inputs ----
            # The gpsimd (SWDGE) queue is FIFO: row gathers go first there;
            # zero-fill and broadcasts are deferred until the gathers issued.
            ident = constp.tile([P, P], bf16)
            nc.sync.dma_start(ident[:], ident_d[:])
            idx_sb = constp.tile([P, NCOL], i32)
            nc.sync.dma_start(idx_sb[:], idx_d[:])
            b1_sb = constp.tile([P, H // P], f32)
            nc.sync.dma_start(b1_sb[:], b1_d[:])
            b2_sb = constp.tile([1, O], bf16)
            nc.sync.dma_start(b2_sb[:], b2_d[:])

            # ---- gather all selected rows into one landing tile ----
            # A single wide tile avoids staging-slot starvation: all NG
            # indirect DMAs stream back-to-back on the gpsimd queue while
            # the PE consumes earlier columns.
            xg_all = constp.tile([P, NG * D], bf16)
            for g in range(NG):
                nc.gpsimd.indirect_dma_start(
                    out=xg_all[:, g * D : (g + 1) * D],
                    out_offset=None,
                    in_=x_d[:],
                    in_offset=bass.IndirectOffsetOnAxis(
                        ap=idx_sb[:, g : g + 1], axis=0
                    ),
                )

            # ---- weights (resident, bf16) ----
            w1_sb = [
                w1pool.tile([P, H], bf16, tag="w1", name=f"w1sb{k}")
                for k in range(KD)
            ]
            for k in range(KD):
                nc.sync.dma_start(w1_sb[k][:], w1_d[k * P : (k + 1) * P, :])
            w2_sb = [
                w2pool.tile([P, O], bf16, tag="w2", name=f"w2sb{k}")
                for k in range(KH)
            ]
            for k in range(KH):
                nc.sync.dma_start(w2_sb[k][:], w2_d[k * P : (k + 1) * P, :])

            # Chunks of whole gather-tiles (up to 3 x 128 = 384 columns).
            GPC = 3  # gather tiles per chunk
            gchunks = [(g0, min(GPC, NG - g0)) for g0 in range(0, NG, GPC)]

            first = True
            for g0, ng in gchunks:
                cw = ng * P
                # transpose chunk columns into X^T layout
                xt_c = [
                    xtp.tile([P, GPC * P], bf16, tag="xt", name=f"xt{g0}_{j}")
                    for j in range(KD)
                ]
                for gl in range(ng):
                    for j in range(KD):
                        tp = pst.tile([P, P], bf16, tag="tp", name=f"tp{g0+gl}_{j}")
                        nc.tensor.transpose(
                            tp[:],
                            xg_all[:, (g0 + gl) * D + j * P : (g0 + gl) * D + (j + 1) * P],
                            ident[:],
                        )
                        nc.vector.tensor_copy(
                            xt_c[j][:, gl * P : (gl + 1) * P], tp[:]
                        )

                # layer 1: H^T chunk = gelu(W1^T @ X^T + b1)
                ht_c = [
                    htp.tile([P, GPC * P], bf16, tag="ht", name=f"ht{g0}_{h}")
                    for h in range(KH)
                ]
                for h in range(KH):
                    acc = ps1.tile([P, GPC * P], f32, tag="l1acc", name=f"l1a{g0}_{h}")
                    for k in range(KD):
                        nc.tensor.matmul(
                            acc[:, :cw],
                            w1_sb[k][:, h * P : (h + 1) * P],
                            xt_c[k][:, :cw],
                            start=(k == 0),
                            stop=(k == KD - 1),
                        )
                    nc.scalar.activation(
                        ht_c[h][:, :cw],
                        acc[:, :cw],
                        act_fn,
                        bias=b1_sb[:, h : h + 1],
                    )

                if first:
                    # gpsimd queue has drained the gathers by now: issue the
                    # zero-fill of unselected rows; overlaps layer-1 compute.
                    first = False
                    zero_sb = constp.tile([P, O], f32)
                    nc.vector.memset(zero_sb[:], 0.0)
                    b2_rep = constp.tile([P, O], bf16)
                    nc.gpsimd.partition_broadcast(b2_rep[:], b2_sb[:])
                    for z in range(NGZ):
                        nc.gpsimd.indirect_dma_start(
                            out=out_d[:],
                            out_offset=bass.IndirectOffsetOnAxis(
                                ap=idx_sb[:, 2 * NG + z : 2 * NG + z + 1], axis=0
                            ),
                            in_=zero_sb[:],
                            in_offset=None,
                            bounds_check=RX - 1,
                            oob_is_err=False,
                        )

                # layer 2 for this chunk's row tiles, batch-major, + scatter
                for rl in range(ng):
                    r = g0 + rl
                    out_sb = outp.tile([P, O], f32, tag="outsb", name=f"osb{r}")
                    for oc in range(2):
                        acc2 = ps2.tile([P, 512], f32, tag="l2acc", name=f"l2a{r}_{oc}")
                        for k in range(KH):
                            nc.tensor.matmul(
                                acc2[:],
                                ht_c[k][:, rl * P : (rl + 1) * P],
                                w2_sb[k][:, oc * 512 : (oc + 1) * 512],
                                start=(k == 0),
                                stop=(k == KH - 1),
                            )
                        nc.vector.tensor_add(
                            out_sb[:, oc * 512 : (oc + 1) * 512],
                            acc2[:],
                            b2_rep[:, oc * 512 : (oc + 1) * 512],
                        )
                    nc.gpsimd.indirect_dma_start(
                        out=out_d[:],
                        out_offset=bass.IndirectOffsetOnAxis(
                            ap=idx_sb[:, NG + r : NG + r + 1], axis=0
                        ),
                        in_=out_sb[:],
                        in_offset=None,
                        bounds_check=RX - 1,
                        oob_is_err=False,
                    )

    nc.compile()
    return nc


def _get_graph(NG, NGZ, EX, act="gelu"):
    key = (NG, NGZ, EX, act)
    if key not in _GRAPH_CACHE:
        _GRAPH_CACHE[key] = _build_graph(NG, NGZ, EX, act)
    return _GRAPH_CACHE[key]


def _pack_idx(rows, n_tiles, pad_mode, oob):
    """rows -> [128, n_tiles] int32, column g = rows[g*128:(g+1)*128]."""
    cap = n_tiles * P
    if pad_mode == "dup":
        pad = rows[-1] if len(rows) else 0
    else:
        pad = oob
    padded = np.full(cap, pad, dtype=np.int64)
    padded[: len(rows)] = np.asarray(rows, dtype=np.int64)
    return np.ascontiguousarray(padded.reshape(n_tiles, P).T).astype(np.int32)


def prepare(input_data, selection_mask, W1, b1, W2, b2, modality_idx, act="gelu"):
    """Host-side sharding/routing prep. Returns (nc, in_maps) or None if no
    rows are selected (output is all zeros)."""
    x = np.asarray(input_data, dtype=np.float32)
    mask = np.asarray(selection_mask, dtype=np.float32)
    midx = int(np.asarray(modality_idx))
    sel = mask[:, midx] > 0.5

    sel_rows = [np.nonzero(sel[i * R : (i + 1) * R])[0] for i in range(NCORES)]
    unsel_rows = [np.nonzero(~sel[i * R : (i + 1) * R])[0] for i in range(NCORES)]
    total_sel = sum(len(r) for r in sel_rows)
    max_unsel = max(len(r) for r in unsel_rows)
    if total_sel == 0:
        return None

    # Load-balance: cores above the uniform target donate their tail rows to
    # a shared "extra" block; cores below fill up from it. The extra block is
    # appended to every core's x shard, so one gather path serves both; the
    # borrowed results come back through extra output rows and the host puts
    # the handful of them (~tens of rows) in place.
    T = -(-total_sel // NCORES)
    own = []
    pool = []  # (owner core, local row) per donated row, in order
    for i in range(NCORES):
        own.append(list(sel_rows[i][:T]))
        pool.extend((i, int(r)) for r in sel_rows[i][T:])
    EX = len(pool)
    assign = {}  # pool idx -> computing core
    gather_lists = []
    p = 0
    for i in range(NCORES):
        g = list(own[i])
        while len(g) < T and p < len(pool):
            g.append(R + p)
            assign[p] = i
            p += 1
        gather_lists.append(g)
    NG = -(-T // P)
    NGZ = -(-max_unsel // P)
    oob = R + EX

    nc = _get_graph(NG, NGZ, EX, act)

    bf = ml_dtypes.bfloat16
    w1_b = np.asarray(W1, dtype=np.float32).astype(bf)
    w2_b = np.asarray(W2, dtype=np.float32).astype(bf)
    b1p = np.ascontiguousarray(
        np.asarray(b1, dtype=np.float32).reshape(H // P, P).T
    )
    b2row = np.asarray(b2, dtype=np.float32).reshape(1, O).astype(bf)
    identm = np.eye(P, dtype=bf)

    x_extra = x[[pool[e][0] * R + pool[e][1] for e in range(EX)]].astype(bf)

    in_maps = []
    for i in range(NCORES):
        idx = np.concatenate(
            [
                _pack_idx(gather_lists[i], NG, "dup", oob),
                _pack_idx(gather_lists[i], NG, "oob", oob),
                _pack_idx(unsel_rows[i], NGZ, "oob", oob),
            ],
            axis=1,
        )
        in_maps.append(
            {
                "x": np.concatenate(
                    [x[i * R : (i + 1) * R].astype(bf), x_extra], axis=0
                ),
                "w1": w1_b,
                "w2": w2_b,
                "b1p": b1p,
                "b2row": b2row,
                "idx": idx,
                "identm": identm,
            }
        )
    # host fixups: (computing core, extra slot, owner core, owner-local row)
    fixups = [(assign[e], e, pool[e][0], pool[e][1]) for e in range(EX)]
    return nc, in_maps, fixups


def kernel(input_data, selection_mask, W1, b1, W2, b2, modality_idx):
    prep = prepare(input_data, selection_mask, W1, b1, W2, b2, modality_idx)
    if prep is None:
        return np.zeros((B, O), dtype=np.float32)
    nc, in_maps, fixups = prep

    from concourse.bass_utils import run_bass_kernel_spmd

    res = run_bass_kernel_spmd(nc, in_maps, core_ids=list(range(NCORES)))
    out = np.concatenate(
        [res.results[i]["out"][:R] for i in range(NCORES)], axis=0
    ).astype(np.float32)
    for comp_core, slot, owner, local_row in fixups:
        out[owner * R + local_row] = res.results[comp_core]["out"][R + slot]
    return np.ascontiguousarray(out)



# revision 2
# speedup vs baseline: 1.1731x; 1.1731x over previous
"""Trainium2 Bass kernel for nn_AdaptiveModalityEncoder.

Reference computation (per row r of input_data [B, D]):
    sel[r] = selection_mask[r, modality_idx] > 0.5
    out[r] = sel[r] ? gelu(x[r] @ W1 + b1) @ W2 + b2 : 0

Strategy (moe_routing, data-parallel across 8 cores):
  - Host computes the selected-row list, gathers + transposes the selected
    rows (routing metadata/prep), and splits them evenly across the 8
    cores; each core runs a pure dense 2-layer MLP in bf16 (fp32
    accumulate) over its T rows and writes a compact batch-major output.
    Host scatters the compact outputs into the zero-filled full output.
  - Device kernel is gather/scatter-free: only linear DMAs, which stripe
    across all 16 HW DMA queues (aggregate ~358 GB/s), so the PE starts
    within ~1 us and runs back-to-back matmuls (PE-bound regime).

Matmul layout: activations feature-major for L1 (X^T tiles [D_part, rows],
pre-transposed on host; H^T = W1^T @ X^T with W1 repacked on host so each
h-tile's 8 k-slices are one contiguous 256 KB load). L2 flips operands so
the output comes out batch-major (OUT = (HT)^T @ W2, lhsT = HT slices,
W2 natural layout) and is written straight out with plain DMAs.
"""

import sys

sys.path.insert(0, "/opt/trn_rl_repo")

import numpy as np
import ml_dtypes

# Problem constants (hardcoded per harness contract).
B, D, H, O, K = 16384, 1024, 2048, 1024, 4
NCORES = 8
P = 128
KD = D // P  # 8 k-tiles for layer 1
KH = H // P  # 16 k-tiles for layer 2

_GRAPH_CACHE = {}


def _build_graph(NG, act="gelu"):
    """Build + compile the per-core Bass graph. NG = number of 128-row
    tiles per core (CP = NG*128 padded rows). Same graph on all 8 cores."""
    import concourse.mybir as mybir
    import concourse.tile as tile
    from concourse import bacc

    f32 = mybir.dt.float32
    bf16 = mybir.dt.bfloat16
    act_fn = {
        "gelu": mybir.ActivationFunctionType.Gelu_apprx_tanh,
        "tanh": mybir.ActivationFunctionType.Tanh,  # CoreSim stand-in
    }[act]

    CP = NG * P  # padded rows per core

    # Column chunks for L1 (PSUM bank = 512 fp32).
    chunks = []
    c0 = 0
    while c0 < CP:
        w = min(512, CP - c0)
        chunks.append((c0, w))
        c0 += w

    nc = bacc.Bacc("TRN2", target_bir_lowering=False, debug=False, num_devices=NCORES)

    xt_d = nc.dram_tensor("xt", [D, CP], bf16, kind="ExternalInput")
    # W1 repacked host-side: block (h, k) at cols (h*KD + k)*P, i.e. one
    # h-tile = one contiguous [P, KD*P] load feeding a full k-chain.
    w1r_d = nc.dram_tensor("w1r", [P, KH * KD * P], bf16, kind="ExternalInput")
    w2_d = nc.dram_tensor("w2", [H, O], bf16, kind="ExternalInput")
    b1p_d = nc.dram_tensor("b1p", [P, KH], f32, kind="ExternalInput")
    b2r_d = nc.dram_tensor("b2r", [P, O], bf16, kind="ExternalInput")
    out_d = nc.dram_tensor("out", [CP, O], f32, kind="ExternalOutput")

    with tile.TileContext(nc) as tc:
        with (
            tc.tile_pool(name="w1pool", bufs=KH) as w1pool,
            tc.tile_pool(name="w2pool", bufs=KH) as w2pool,
            tc.tile_pool(name="xtp", bufs=KD) as xtp,
            tc.tile_pool(name="htp", bufs=KH) as htp,
            tc.tile_pool(name="outp", bufs=4) as outp,
            tc.tile_pool(name="const", bufs=1) as constp,
            tc.tile_pool(name="ps1", bufs=3, space="PSUM") as ps1,  # layer 1
            tc.tile_pool(name="ps2", bufs=3, space="PSUM") as ps2,  # layer 2
        ):
            # ---- DMA issue order = queue order: critical-path first ----
            b1_sb = constp.tile([P, KH], f32)
            nc.sync.dma_start(b1_sb[:], b1p_d[:])

            w1_sb = [
                w1pool.tile([P, KD * P], bf16, tag="w1", name=f"w1sb{h}")
                for h in range(KH)
            ]
            # First two h-tiles of W1 ahead of X^T so the very first matmul
            # chain is fed immediately; the rest after X^T.
            for h in range(2):
                nc.sync.dma_start(
                    w1_sb[h][:], w1r_d[:, h * KD * P : (h + 1) * KD * P]
                )

            xt_sb = [
                xtp.tile([P, CP], bf16, tag="xt", name=f"xtsb{k}")
                for k in range(KD)
            ]
            for k in range(KD):
                nc.sync.dma_start(xt_sb[k][:], xt_d[k * P : (k + 1) * P, :])

            for h in range(2, KH):
                nc.sync.dma_start(
                    w1_sb[h][:], w1r_d[:, h * KD * P : (h + 1) * KD * P]
                )

            w2_sb = [
                w2pool.tile([P, O], bf16, tag="w2", name=f"w2sb{k}")
                for k in range(KH)
            ]
            for k in range(KH):
                nc.sync.dma_start(w2_sb[k][:], w2_d[k * P : (k + 1) * P, :])

            b2_sb = constp.tile([P, O], bf16)
            nc.sync.dma_start(b2_sb[:], b2r_d[:])

            ht_sb = [
                htp.tile([P, CP], bf16, tag="ht", name=f"htsb{h}")
                for h in range(KH)
            ]

            # ---- compute: per column chunk, L1 then L2 ----
            for ci, (c0, cw) in enumerate(chunks):
                # layer 1: H^T chunk = gelu(W1^T @ X^T + b1)
                for h in range(KH):
                    acc = ps1.tile([P, cw], f32, tag="l1acc", name=f"l1a{ci}_{h}")
                    for k in range(KD):
                        nc.tensor.matmul(
                            acc[:],
                            w1_sb[h][:, k * P : (k + 1) * P],
                            xt_sb[k][:, c0 : c0 + cw],
                            start=(k == 0),
                            stop=(k == KD - 1),
                        )
                    nc.scalar.activation(
                        ht_sb[h][:, c0 : c0 + cw],
                        acc[:],
                        act_fn,
                        bias=b1_sb[:, h : h + 1],
                    )

                # layer 2, batch-major: OUT rows = (HT slice)^T @ W2 + b2
                for rl in range(cw // P):
                    r0 = c0 + rl * P
                    for oc in range(2):
                        acc2 = ps2.tile(
                            [P, 512], f32, tag="l2acc", name=f"l2a{ci}_{rl}_{oc}"
                        )
                        for k in range(KH):
                            nc.tensor.matmul(
                                acc2[:],
                                ht_sb[k][:, r0 : r0 + P],
                                w2_sb[k][:, oc * 512 : (oc + 1) * 512],
                                start=(k == 0),
                                stop=(k == KH - 1),
                            )
                        ob = outp.tile(
                            [P, 512], f32, tag="outsb", name=f"osb{ci}_{rl}_{oc}"
                        )
                        nc.vector.tensor_add(
                            ob[:], acc2[:], b2_sb[:, oc * 512 : (oc + 1) * 512]
                        )
                        nc.sync.dma_start(
                            out_d[r0 : r0 + P, oc * 512 : (oc + 1) * 512], ob[:]
                        )

    nc.compile()
    return nc


def _get_graph(NG, act="gelu"):
    key = (NG, act)
    if key not in _GRAPH_CACHE:
        _GRAPH_CACHE[key] = _build_graph(NG, act)
    return _GRAPH_CACHE[key]


def prepare(input_data, selection_mask, W1, b1, W2, b2, modality_idx, act="gelu"):
    """Host-side routing/sharding prep. Returns (nc, in_maps, meta) or None
    if no rows are selected (output is all zeros)."""
    x = np.asarray(input_data, dtype=np.float32)
    mask = np.asarray(selection_mask, dtype=np.float32)
    midx = int(np.asarray(modality_idx))
    rows = np.nonzero(mask[:, midx] > 0.5)[0]
    total = len(rows)
    if total == 0:
        return None

    T = -(-total // NCORES)  # rows per core
    NG = -(-T // P)
    CP = NG * P

    nc = _get_graph(NG, act)

    bf = ml_dtypes.bfloat16
    x_bf = x.astype(bf)
    w1r = np.ascontiguousarray(
        np.asarray(W1, dtype=np.float32)
        .astype(bf)
        .reshape(KD, P, KH, P)
        .transpose(1, 2, 0, 3)
        .reshape(P, KH * KD * P)
    )
    w2_b = np.asarray(W2, dtype=np.float32).astype(bf)
    b1p = np.ascontiguousarray(
        np.asarray(b1, dtype=np.float32).reshape(KH, P).T
    )
    b2r = np.ascontiguousarray(
        np.broadcast_to(np.asarray(b2, dtype=np.float32).astype(bf), (P, O))
    )

    # Pad the global selected-row list to NCORES*CP; padding rows compute
    # garbage that the host scatter ignores.
    rows_pad = np.concatenate(
        [rows, np.full(NCORES * CP - total, rows[-1], dtype=rows.dtype)]
    )

    in_maps = []
    for i in range(NCORES):
        r_i = rows_pad[i * CP : (i + 1) * CP]
        xt = np.ascontiguousarray(x_bf[r_i].T)
        in_maps.append(
            {"xt": xt, "w1r": w1r, "w2": w2_b, "b1p": b1p, "b2r": b2r}
        )
    return nc, in_maps, (rows, total, CP)


def _assemble(res, meta):
    rows, total, CP = meta
    compact = np.concatenate(
        [res.results[i]["out"] for i in range(NCORES)], axis=0
    )[:total]
    out = np.zeros((B, O), dtype=np.float32)
    out[rows] = compact
    return out


def run_full(inputs, trace=False):
    """Shared by kernel() and test harness: returns (out, res)."""
    prep = prepare(**inputs)
    if prep is None:
        return np.zeros((B, O), dtype=np.float32), None
    nc, in_maps, meta = prep

    from concourse.bass_utils import run_bass_kernel_spmd

    res = run_bass_kernel_spmd(
        nc, in_maps, core_ids=list(range(NCORES)), trace=trace
    )
    return _assemble(res, meta), res


def kernel(input_data, selection_mask, W1, b1, W2, b2, modality_idx):
    out, _ = run_full(
        dict(
            input_data=input_data,
            selection_mask=selection_mask,
            W1=W1,
            b1=b1,
            W2=W2,
            b2=b2,
            modality_idx=modality_idx,
        )
    )
    return out
